# revision 13
# baseline (speedup 1.0000x reference)
"""DifferentialAttentionBlock on 8 NeuronCores.

Sharding: DP on batch (cores 0-3 = batch 0, 4-7 = batch 1) x TP on heads
(4 heads per core) through attention; then a 4-rank AllGather of bf16
attnT shards WITHIN the batch group, chunked over q (big chunk first) so
the collective and the Wo matmuls overlap attention compute.  Each core
produces 256 Wo output columns for its own batch.

Per-core dataflow (transposed-activation layout):
  qT/kT/vT (host-transposed bf16) -> projections q1T/q2T/k1T/k2T
  [128c, S] bf16 and vv [S, 256] (+ones col) -> per-head transposed
  scores as K=32 PE-tile matmuls (no zero padding), both differential
  branches packed in one [128, 1024] PSUM tile -> single exp per k-tile
  (ACT, scale=1/8, max-free, [128,2,n] strided AP) -> bf16 A@V in outT
  form (vv stationary; fused colsum row) for both branches ->
  colsum rows copied to SBUF (ACT), PE K=1 broadcast matmuls (ones and
  1/lambda stationary rows) -> one DVE reciprocal + 2 mults + subtract
  per head -> attnT bf16 chunk -> 4-rank AllGather -> Wo col-shard
  matmul -> out [256, S] per core.  Output bias bo on host.
"""

import math
import numpy as np

B, S, D = 2, 1024, 1024
H = 16
DH = 32          # q/k half head dim
DK = 64          # v head dim
HPC = 4          # heads per core
CPB = 4          # cores per batch (TP group size)
NCORES = 8
LAMBDA_INIT = 0.8 - 0.6 * math.exp(-0.3 * (1 - 1))
NSK = S // 128   # 8 s_k tiles
CHW = 512        # sq chunk width
NCH = S // CHW   # 2 chunks
RG8 = [list(range(8))]
RG4 = [[0, 1, 2, 3], [4, 5, 6, 7]]

PROFILE = False
LAST_EXEC_NS = None
LAST_RESULTS = None

_cache = {}


def _try_install_ntff_hook():
    try:
        import sys, types
        import antenv
        try:
            import antenv.axon_hooks  # noqa: F401
            return
        except ImportError:
            pass
        mod = types.ModuleType("antenv.axon_hooks")
        mod._hook = None
        mod.set_axon_ntff_profile_hook = lambda h: setattr(mod, "_hook", h)
        mod.get_axon_ntff_profile_hook = lambda: mod._hook
        sys.modules["antenv.axon_hooks"] = mod
        antenv.axon_hooks = mod
        from trn_agent_boot.trn_boot import _ntff_profile_via_ctypes
        mod._hook = _ntff_profile_via_ctypes('/opt/axon/libaxon_pjrt.so')
    except Exception:
        pass


def _build(causal: bool):
    import concourse.bacc as bacc
    import concourse.mybir as mybir
    import concourse.tile as tile
    from concourse.tile_rust import add_dep_helper

    dt = mybir.dt
    f32, f32r, bf16 = dt.float32, dt.float32r, dt.bfloat16
    AF = mybir.ActivationFunctionType
    OP = mybir.AluOpType

    nc = bacc.Bacc("TRN2", target_bir_lowering=False, debug=False,
                   num_devices=NCORES)

    def inp(name, shape, d=f32):
        return nc.dram_tensor(name, shape, d, kind="ExternalInput")

    qT = inp("qT", [D, S], bf16)
    kT = inp("kT", [D, S], bf16)
    vT = inp("vT", [D, S], bf16)
    Wq1 = inp("Wq1", [D, 128], bf16);  Wq2 = inp("Wq2", [D, 128], bf16)
    Wk1 = inp("Wk1", [D, 128], bf16);  Wk2 = inp("Wk2", [D, 128], bf16)
    Wv = inp("Wv", [D, 256], bf16)
    Wob = inp("Wob", [D, 256], bf16)          # my 256 output columns
    bq1 = inp("bq1", [128, 1]);  bq2 = inp("bq2", [128, 1])
    bk1 = inp("bk1", [128, 1]);  bk2 = inp("bk2", [128, 1])
    bv = inp("bv", [1, 256], bf16)
    ones_in = inp("ones1", [1, 128], bf16)
    dtriu = inp("dtriu", [128, 256], bf16)    # [triu | triu]
    bcc = inp("bcc", [128, 128], bf16)        # r64: 1s||0s, r65: 0s||lam
    maskT = None if causal else inp("maskT", [S, S])
    out_ext = nc.dram_tensor("out", [256, S], f32, kind="ExternalOutput")

    with tile.TileContext(nc) as tc:
        with (
            tc.tile_pool(name="const", bufs=1) as cpool,
            tc.tile_pool(name="wts", bufs=1) as wpool,
            tc.tile_pool(name="proj", bufs=1) as ppool,
            tc.tile_pool(name="acts", bufs=2) as apool,
            tc.tile_pool(name="edata", bufs=2) as epool,
            tc.tile_pool(name="small", bufs=2) as spool,
            tc.tile_pool(name="outs", bufs=2) as opool,
            tc.tile_pool(name="dram", bufs=1, space="DRAM") as dpool,
        ):
            # tiny AllGather issued first: absorbs cross-core launch skew
            # while the load phase runs, so real collectives see synced peers
            dummy_in = dpool.tile([1, 16], bf16, name="dummy_in")
            dummy_out = dpool.tile([8, 16], bf16, name="dummy_out")
            nc.gpsimd.dma_start(dummy_in[:], dtriu[0:1, 0:16])
            nc.gpsimd.collective_compute(
                "AllGather", mybir.AluOpType.bypass, replica_groups=RG8,
                ins=[dummy_in.opt()], outs=[dummy_out.opt()])

            # ---- big loads all on the sync DGE, in dependency order ----
            wsb = {}
            for name, t in (("Wq1", Wq1), ("Wq2", Wq2)):
                wsb[name] = wpool.tile([128, 8 * 128], bf16, tag=name,
                                       name=name)
                nc.sync.dma_start(
                    wsb[name][:].rearrange("p (d c) -> p d c", d=8),
                    t.rearrange("(d p) c -> p d c", p=128))
            qblk = []
            for g in range(2):
                qb = apool.tile([128, 4 * S], bf16, tag="qTd",
                                name=f"qblk{g}")
                nc.sync.dma_start(
                    qb[:].rearrange("p (d x) -> p d x", d=4),
                    qT[g * 512:(g + 1) * 512, :]
                    .rearrange("(d p) x -> p d x", p=128))
                qblk.append(qb)
            for name, t in (("Wk1", Wk1), ("Wk2", Wk2)):
                wsb[name] = wpool.tile([128, 8 * 128], bf16, tag=name,
                                       name=name)
                nc.sync.dma_start(
                    wsb[name][:].rearrange("p (d c) -> p d c", d=8),
                    t.rearrange("(d p) c -> p d c", p=128))
            kblk = []
            vtiles = []
            for g in range(2):
                kb = apool.tile([128, 4 * S], bf16, tag="kTd",
                                name=f"kblk{g}")
                nc.sync.dma_start(
                    kb[:].rearrange("p (d x) -> p d x", d=4),
                    kT[g * 512:(g + 1) * 512, :]
                    .rearrange("(d p) x -> p d x", p=128))
                kblk.append(kb)
            for g in range(2):
                vb = apool.tile([128, 4 * S], bf16, tag="vTd",
                                name=f"vblk{g}")
                nc.sync.dma_start(
                    vb[:].rearrange("p (d x) -> p d x", d=4),
                    vT[g * 512:(g + 1) * 512, :]
                    .rearrange("(d p) x -> p d x", p=128))
                vtiles.append(vb)
            wv_sb = wpool.tile([128, 8 * 256], bf16, tag="Wv")
            nc.sync.dma_start(wv_sb[:].rearrange("p (d c) -> p d c", d=8),
                              Wv.rearrange("(d p) c -> p d c", p=128))
            wo_sb = wpool.tile([128, 8 * 256], bf16, tag="Wob")
            nc.sync.dma_start(wo_sb[:].rearrange("p (d c) -> p d c", d=8),
                              Wob.rearrange("(d p) c -> p d c", p=128))

            # ---- small consts on the scalar DGE ----
            dtriu_sb = cpool.tile([128, 256], bf16, tag="dtriu")
            nc.scalar.dma_start(dtriu_sb[:], dtriu[:, :])
            bcc_sb = cpool.tile([128, 128], bf16, tag="bcc")
            nc.scalar.dma_start(bcc_sb[:], bcc[:, :])
            ones1 = cpool.tile([1, 128], bf16, tag="ones1")
            nc.scalar.dma_start(ones1[:], ones_in[:, :])
            bsb = {}
            for name, t in (("bq1", bq1), ("bq2", bq2), ("bk1", bk1),
                            ("bk2", bk2)):
                bsb[name] = cpool.tile([128, 1], f32, tag=name, name=name)
                nc.scalar.dma_start(bsb[name][:], t[:, :])
            bv_sb = cpool.tile([1, 256], bf16, tag="bv")
            nc.scalar.dma_start(bv_sb[:], bv[:, :])

            # ---- phase B: q then k projections ----
            # Each projection lands as TWO [64, S] tiles (heads 0-1, 2-3)
            # so per-head 32-row slices sit at matmul-legal bases {0, 32}.
            def split_tiles(pfx):
                return [ppool.tile([64, S], bf16, tag=f"{pfx}{j}",
                                   name=f"{pfx}{j}") for j in range(2)]
            q1T = split_tiles("q1T"); q2T = split_tiles("q2T")
            k1T = split_tiles("k1T"); k2T = split_tiles("k2T")

            def proj_pair(pool_name, blk, w1, w2, b1, b2, o1T, o2T):
                with tc.tile_pool(name=pool_name, bufs=1,
                                  space="PSUM") as psB:
                    p1 = psB.tile([128, S], f32, tag="p1", name=pool_name + "1")
                    p2 = psB.tile([128, S], f32, tag="p2", name=pool_name + "2")
                    for d in range(8):
                        xTd = blk[d // 4][:, (d % 4) * S:(d % 4 + 1) * S]
                        for ps, wname in ((p1, w1), (p2, w2)):
                            lhsT = wsb[wname][:, d * 128:(d + 1) * 128]
                            for half in range(2):
                                nc.tensor.matmul(
                                    ps[:, half * 512:(half + 1) * 512], lhsT,
                                    xTd[:, half * 512:(half + 1) * 512],
                                    start=(d == 0), stop=(d == 7))
                    # aligned halves on ACT, shifted halves on DVE
                    nc.scalar.activation(o1T[0][:], p1[0:64, :], AF.Identity,
                                         bias=bsb[b1][0:64, :])
                    nc.vector.tensor_scalar(o1T[1][:], p1[64:128, :],
                                            bsb[b1][64:128, :], None, OP.add)
                    nc.scalar.activation(o2T[0][:], p2[0:64, :], AF.Identity,
                                         bias=bsb[b2][0:64, :])
                    nc.vector.tensor_scalar(o2T[1][:], p2[64:128, :],
                                            bsb[b2][64:128, :], None, OP.add)

            # phase C PSUM (4 banks) coexists with the 4-bank proj pools
            # so attention's pools never wait on the v-projection drain.
            vvo = ppool.tile([128, 8 * 260], bf16, tag="vvo")
            with tc.tile_pool(name="psC", bufs=1, space="PSUM") as psC:
                proj_pair("psBq", qblk, "Wq1", "Wq2", "bq1", "bq2", q1T, q2T)
                proj_pair("psBk", kblk, "Wk1", "Wk2", "bk1", "bk2", k1T, k2T)

                # ---- vv projection in two rounds of 4 s-tiles ----
                for rnd in range(2):
                    pr = [psC.tile([128, 256], f32, tag=f"vv{j}",
                                   name=f"vv{rnd}{j}") for j in range(4)]
                    for d in range(8):
                        vTd = vtiles[d // 4][:, (d % 4) * S:(d % 4 + 1) * S]
                        for j in range(4):
                            i = 4 * rnd + j
                            nc.tensor.matmul(
                                pr[j][:], vTd[:, i * 128:(i + 1) * 128],
                                wv_sb[:, d * 256:(d + 1) * 256],
                                start=(d == 0), stop=False)
                    for j in range(4):
                        i = 4 * rnd + j
                        nc.tensor.matmul(pr[j][:], ones1[:], bv_sb[:],
                                         start=False, stop=True)
                        blk = vvo[:, i * 260:(i + 1) * 260]
                        blk3 = blk.rearrange("p (h c) -> p h c", c=65)
                        nc.vector.tensor_copy(
                            blk3[:, :, 0:64],
                            pr[j].rearrange("p (h c) -> p h c", c=64))
                        nc.vector.memset(blk3[:, :, 64:65], 1.0)

            # ---- phase D: attention, chunked over q; AG + Wo per chunk ----
            qproj = (q1T, q2T)
            kproj = (k1T, k2T)
            dtriu3 = dtriu_sb[:].rearrange("p (m x) -> p m x", m=2)
            with (
                tc.tile_pool(name="psS", bufs=2, space="PSUM") as psS,
                tc.tile_pool(name="psO", bufs=2, space="PSUM") as psO,
                tc.tile_pool(name="psP", bufs=1, space="PSUM") as psP,
                tc.tile_pool(name="psW", bufs=1, space="PSUM") as psW,
                tc.tile_pool(name="mloc", bufs=2) as mpool,
            ):
                ship = {}
                for c in (1, 0):
                    cs = c * CHW
                    ilist = list(range(min(NSK, (cs + CHW) // 128))) \
                        if causal else list(range(NSK))
                    if not causal:
                        mT = {}
                        for i in ilist:
                            mT[i] = mpool.tile([128, CHW], f32,
                                               tag=f"mT{i}",
                                               name=f"mT{c}{i}")
                            nc.sync.dma_start(
                                mT[i][:],
                                maskT[i * 128:(i + 1) * 128, cs:cs + CHW])
                    aTt = [spool.tile([128, CHW], bf16, tag=f"aT{kk}",
                                      name=f"aT{kk}c{c}")
                           for kk in range(2)]
                    for h in range(4):
                        hj = h // 2
                        hp = slice(32 * (h % 2), 32 * (h % 2) + 32)
                        etiles = []
                        nx = len(ilist)
                        o0 = psO.tile([128, CHW], f32, tag="o",
                                      name=f"o0{c}{h}")
                        o1 = psO.tile([128, CHW], f32, tag="o",
                                      name=f"o1{c}{h}")

                        def av(x):
                            i, e, off = etiles[x]
                            lhsT = vvo[:, 260 * i + 65 * h:
                                       260 * i + 65 * h + 65]
                            nc.tensor.matmul(
                                o0[0:65, off:CHW], lhsT, e[:, off:CHW],
                                start=(x == 0), stop=(x == nx - 1))
                            nc.tensor.matmul(
                                o1[0:65, off:CHW], lhsT,
                                e[:, CHW + off:2 * CHW],
                                start=(x == 0), stop=(x == nx - 1))

                        for x, i in enumerate(ilist):
                            lo = max(cs, 128 * i) if causal else cs
                            n = cs + CHW - lo
                            off = lo - cs
                            ps = psS.tile([128, 2 * CHW], f32, tag="s",
                                          name=f"s{c}{h}{i}")
                            for m in range(2):
                                nc.tensor.matmul(
                                    ps[:, m * CHW:m * CHW + n],
                                    kproj[m][hj][hp, i * 128:(i + 1) * 128],
                                    qproj[m][hj][hp, lo:cs + CHW],
                                    start=True, stop=True)
                            if not causal:
                                for m in range(2):
                                    nc.vector.tensor_tensor(
                                        ps[:, m * CHW:m * CHW + n],
                                        ps[:, m * CHW:m * CHW + n],
                                        mT[i][:, 0:n], OP.add)
                            e = epool.tile([128, 2 * CHW], bf16,
                                           tag=f"e{i}",
                                           name=f"e{c}h{h}i{i}")
                            e3 = e[:].rearrange("p (m x) -> p m x", m=2)
                            ps3 = ps[:].rearrange("p (m x) -> p m x", m=2)
                            nc.scalar.activation(
                                e3[:, :, off:CHW], ps3[:, :, 0:n], AF.Exp,
                                scale=0.125)
                            if causal and 128 * i >= cs:
                                nc.vector.tensor_tensor(
                                    e3[:, :, off:off + 128],
                                    e3[:, :, off:off + 128],
                                    dtriu3, OP.mult)
                            etiles.append((i, e, off))
                            # AV for the previous tile keeps the PE fed
                            # while ACT runs this tile's exp
                            if x >= 1:
                                av(x - 1)
                        av(nx - 1)
                        # colsum rows -> SBUF (partition-aligned at 64)
                        rs2 = spool.tile([65, 2 * CHW], f32, tag="rs2",
                                         name=f"rs2{c}{h}")
                        nc.scalar.copy(rs2[64:65, 0:CHW], o0[64:65, :])
                        nc.scalar.copy(rs2[64:65, CHW:2 * CHW], o1[64:65, :])
                        # compact reciprocal: rows -> [128, 8] via SBUF DMA,
                        # recip there (recip is ~6.4ns per free elem), back
                        cT = spool.tile([128, 8], f32, tag="cT",
                                        name=f"cT{c}{h}")
                        nc.sync.dma_start(cT[:], rs2[64:65, :])
                        cT2 = spool.tile([128, 8], bf16, tag="cT2",
                                         name=f"cT2{c}{h}")
                        with nc.allow_low_precision(
                                reason="1/colsum as bf16 bcast operand"):
                            nc.vector.reciprocal(cT2[:], cT[:])
                        rrow = spool.tile([66, CHW], bf16, tag="rrow",
                                          name=f"rrow{c}{h}")
                        nc.sync.dma_start(
                            rrow[64:65, :].rearrange("o (p x) -> o p x", p=64),
                            cT2[0:64, :])
                        nc.sync.dma_start(
                            rrow[65:66, :].rearrange("o (p x) -> o p x", p=64),
                            cT2[64:128, :])
                        # single K=2 broadcast: rows 0:64 = 1/s1,
                        # rows 64:128 = lam/s2   (lam folded into bcc row 65)
                        pbc = psP.tile([128, CHW], f32, tag="pbc",
                                       name=f"pbc{c}{h}")
                        nc.tensor.matmul(
                            pbc[:], bcc_sb[64:66, 0:128], rrow[64:66, :],
                            start=True, stop=True)
                        pbcS = spool.tile([128, CHW], f32, tag="pbcS",
                                          name=f"pbcS{c}{h}")
                        nc.vector.tensor_copy(pbcS[:], pbc[:])
                        t0 = spool.tile([64, CHW], f32, tag="t0",
                                        name=f"t0{c}{h}")
                        t1 = spool.tile([64, CHW], f32, tag="t1",
                                        name=f"t1{c}{h}")
                        nc.vector.tensor_tensor(t0[:], o0[0:64, :],
                                                pbcS[0:64, :], OP.mult)
                        nc.vector.tensor_tensor(t1[:], o1[0:64, :],
                                                pbcS[64:128, :], OP.mult)
                        dst = aTt[h // 2][64 * (h % 2):64 * (h % 2) + 64, :]
                        nc.vector.tensor_tensor(dst, t0[:], t1[:],
                                                OP.subtract)
                    # ship this chunk: 4-rank AllGather within batch group
                    bounce = dpool.tile([256, CHW], bf16, name=f"bounce{c}")
                    agc = dpool.tile([CPB * 256, CHW], bf16,
                                     name=f"agc{c}")
                    for kk in range(2):
                        nc.sync.dma_start(
                            bounce[128 * kk:128 * (kk + 1), :], aTt[kk][:])
                    cc = nc.gpsimd.collective_compute(
                        "AllGather", mybir.AluOpType.bypass,
                        replica_groups=RG4,
                        ins=[bounce.opt()], outs=[agc.opt()])
                    ship[c] = (agc, cc, cs)

                # Wo loop AFTER both collectives are issued.  myt/out DMAs
                # go on the gpsimd queue: their long semaphore waits (on the
                # AllGather / Wo copies) must not head-of-line-block the
                # sync queue that carries attention-internal DMAs.
                for c in (1, 0):
                    agc, cc, cs = ship[c]
                    myt = opool.tile([128, 8 * CHW], bf16, tag="myt",
                                     name=f"myt{c}")
                    md = nc.gpsimd.dma_start(
                        myt[:].rearrange("p (k x) -> p k x", k=8),
                        agc.rearrange("(k p) x -> p k x", p=128))
                    add_dep_helper(md.ins, cc.ins, reason="myt after AG")
                    for ch in range(2):
                        wps = psW.tile([128, CHW], f32, tag="wo",
                                       name=f"wo{c}{ch}")
                        for k in range(8):
                            nc.tensor.matmul(
                                wps[:],
                                wo_sb[:, 256 * k + 128 * ch:
                                      256 * k + 128 * ch + 128],
                                myt[:, CHW * k:CHW * (k + 1)],
                                start=(k == 0), stop=(k == 7))
                        osb = opool.tile([128, CHW], f32, tag="osb",
                                         name=f"osb{c}{ch}")
                        if ch == 0:
                            nc.scalar.copy(osb[:], wps[:])
                        else:
                            nc.vector.tensor_copy(osb[:], wps[:])
                        nc.gpsimd.dma_start(
                            out_ext[128 * ch:128 * (ch + 1), cs:cs + CHW],
                            osb[:])

    nc.compile()
    return nc


def kernel(**inputs):
    global LAST_EXEC_NS
    import ml_dtypes

    q = np.asarray(inputs["q"], dtype=np.float32)
    k = np.asarray(inputs["k"], dtype=np.float32)
    v = np.asarray(inputs["v"], dtype=np.float32)
    mask = np.asarray(inputs["mask"])
    f32 = np.float32
    Wq1f = np.asarray(inputs["Wq1"], f32); Wq2f = np.asarray(inputs["Wq2"], f32)
    Wk1f = np.asarray(inputs["Wk1"], f32); Wk2f = np.asarray(inputs["Wk2"], f32)
    Wvf = np.asarray(inputs["Wv"], f32);   Wof = np.asarray(inputs["Wo"], f32)
    bq1f = np.asarray(inputs["bq1"], f32); bq2f = np.asarray(inputs["bq2"], f32)
    bk1f = np.asarray(inputs["bk1"], f32); bk2f = np.asarray(inputs["bk2"], f32)
    bvf = np.asarray(inputs["bv"], f32);   bof = np.asarray(inputs["bo"], f32)
    lam = float(np.exp(float(inputs["lq1"][0]) * float(inputs["lk1"][0]))
                - np.exp(float(inputs["lq2"][0]) * float(inputs["lk2"][0]))
                + LAMBDA_INIT)

    mk = (mask.reshape(B, S, S) != 0)
    causal = bool((mk == np.tril(np.ones((S, S), bool))[None]).all())

    key = "causal" if causal else "general"
    if key not in _cache:
        _cache[key] = _build(causal)
    nc = _cache[key]

    bfl = ml_dtypes.bfloat16
    qT = [np.ascontiguousarray(q[b].T).astype(bfl) for b in range(B)]
    kTl = [np.ascontiguousarray(k[b].T).astype(bfl) for b in range(B)]
    vTl = [np.ascontiguousarray(v[b].T).astype(bfl) for b in range(B)]
    Wob = Wof.astype(bfl)
    triu = np.triu(np.ones((128, 128), f32))
    dtriu = np.concatenate([triu, triu], axis=1).astype(bfl)
    bcc = np.zeros((128, 128), f32)
    bcc[64, 0:64] = 1.0
    bcc[65, 64:128] = lam
    bcc = bcc.astype(ml_dtypes.bfloat16)
    maskTs = None
    if not causal:
        maskTs = [np.ascontiguousarray(
            np.where(mk[b], np.float32(0), np.float32(-1e9)).T)
            for b in range(B)]

    in_maps = []
    for c in range(NCORES):
        b, g = divmod(c, CPB)
        im = dict(
            qT=qT[b], kT=kTl[b], vT=vTl[b],
            Wq1=np.ascontiguousarray(Wq1f[:, 128 * g:128 * (g + 1)]).astype(bfl),
            Wq2=np.ascontiguousarray(Wq2f[:, 128 * g:128 * (g + 1)]).astype(bfl),
            Wk1=np.ascontiguousarray(Wk1f[:, 128 * g:128 * (g + 1)]).astype(bfl),
            Wk2=np.ascontiguousarray(Wk2f[:, 128 * g:128 * (g + 1)]).astype(bfl),
            Wv=np.ascontiguousarray(Wvf[:, 256 * g:256 * (g + 1)]).astype(bfl),
            Wob=np.ascontiguousarray(Wob[:, 256 * g:256 * (g + 1)]),
            bq1=np.ascontiguousarray(bq1f[128 * g:128 * (g + 1)]).reshape(128, 1),
            bq2=np.ascontiguousarray(bq2f[128 * g:128 * (g + 1)]).reshape(128, 1),
            bk1=np.ascontiguousarray(bk1f[128 * g:128 * (g + 1)]).reshape(128, 1),
            bk2=np.ascontiguousarray(bk2f[128 * g:128 * (g + 1)]).reshape(128, 1),
            bv=np.ascontiguousarray(bvf[256 * g:256 * (g + 1)]).reshape(1, 256).astype(bfl),
            dtriu=dtriu, bcc=bcc,
            ones1=np.ones((1, 128), bfl),
        )
        if not causal:
            im["maskT"] = maskTs[b]
        in_maps.append(im)

    from concourse.bass_utils import run_bass_kernel_spmd
    if PROFILE:
        _try_install_ntff_hook()
        res = run_bass_kernel_spmd(nc, in_maps, list(range(NCORES)),
                                   trace=True)
        LAST_EXEC_NS = res.exec_time_ns
        globals()["LAST_RESULTS"] = res
    else:
        res = run_bass_kernel_spmd(nc, in_maps, list(range(NCORES)))

    out = np.empty((B, S, D), np.float32)
    for c in range(NCORES):
        b, g = divmod(c, CPB)
        o = res.results[c]["out"]
        out[b, :, 256 * g:256 * (g + 1)] = o.T
    out += bof[None, None, :]
    return out


# revision 14
# speedup vs baseline: 1.3231x; 1.3231x over previous
"""DifferentialAttentionBlock on 8 NeuronCores.

Sharding: DP on batch (cores 0-3 = batch 0, 4-7 = batch 1) x TP on heads
(4 heads per core) through attention; then a 4-rank AllGather of bf16
attnT shards WITHIN the batch group, chunked over q (big chunk first) so
the collective and the Wo matmuls overlap attention compute.  Each core
produces 256 Wo output columns for its own batch.

Per-core dataflow (transposed-activation layout):
  qT/kT/vT (host-transposed bf16) -> projections q1T/q2T/k1T/k2T
  [128c, S] bf16 and vv [S, 256] (+ones col) -> per-head transposed
  scores as K=32 PE-tile matmuls (no zero padding), both differential
  branches packed in one [128, 1024] PSUM tile -> single exp per k-tile
  (ACT, scale=1/8, max-free, [128,2,n] strided AP) -> bf16 A@V in outT
  form (vv stationary; fused colsum row) for both branches ->
  colsum rows copied to SBUF (ACT), PE K=1 broadcast matmuls (ones and
  1/lambda stationary rows) -> one DVE reciprocal + 2 mults + subtract
  per head -> attnT bf16 chunk -> 4-rank AllGather -> Wo col-shard
  matmul -> out [256, S] per core.  Output bias bo on host.
"""

import math
import numpy as np

B, S, D = 2, 1024, 1024
H = 16
DH = 32          # q/k half head dim
DK = 64          # v head dim
HPC = 4          # heads per core
CPB = 4          # cores per batch (TP group size)
NCORES = 8
LAMBDA_INIT = 0.8 - 0.6 * math.exp(-0.3 * (1 - 1))
NSK = S // 128   # 8 s_k tiles
CHW = 512        # sq chunk width
NCH = S // CHW   # 2 chunks
RG8 = [list(range(8))]
RG4 = [[0, 1, 2, 3], [4, 5, 6, 7]]

PROFILE = False
LAST_EXEC_NS = None
LAST_RESULTS = None

_cache = {}


def _try_install_ntff_hook():
    try:
        import sys, types
        import antenv
        try:
            import antenv.axon_hooks  # noqa: F401
            return
        except ImportError:
            pass
        mod = types.ModuleType("antenv.axon_hooks")
        mod._hook = None
        mod.set_axon_ntff_profile_hook = lambda h: setattr(mod, "_hook", h)
        mod.get_axon_ntff_profile_hook = lambda: mod._hook
        sys.modules["antenv.axon_hooks"] = mod
        antenv.axon_hooks = mod
        from trn_agent_boot.trn_boot import _ntff_profile_via_ctypes
        mod._hook = _ntff_profile_via_ctypes('/opt/axon/libaxon_pjrt.so')
    except Exception:
        pass


def _build(causal: bool):
    import concourse.bacc as bacc
    import concourse.mybir as mybir
    import concourse.tile as tile
    from concourse.tile_rust import add_dep_helper

    dt = mybir.dt
    f32, f32r, bf16 = dt.float32, dt.float32r, dt.bfloat16
    AF = mybir.ActivationFunctionType
    OP = mybir.AluOpType

    nc = bacc.Bacc("TRN2", target_bir_lowering=False, debug=False,
                   num_devices=NCORES)

    def inp(name, shape, d=f32):
        return nc.dram_tensor(name, shape, d, kind="ExternalInput")

    qT = inp("qT", [D, S], bf16)
    kT = inp("kT", [D, S], bf16)
    vT = inp("vT", [D, S], bf16)
    Wq1 = inp("Wq1", [D, 128], bf16);  Wq2 = inp("Wq2", [D, 128], bf16)
    Wk1 = inp("Wk1", [D, 128], bf16);  Wk2 = inp("Wk2", [D, 128], bf16)
    Wv = inp("Wv", [D, 256], bf16)
    Wob = inp("Wob", [D, 256], bf16)          # my 256 output columns
    bq1 = inp("bq1", [128, 1]);  bq2 = inp("bq2", [128, 1])
    bk1 = inp("bk1", [128, 1]);  bk2 = inp("bk2", [128, 1])
    bv = inp("bv", [1, 256], bf16)
    ones_in = inp("ones1", [1, 128], bf16)
    dtriu = inp("dtriu", [128, 256], bf16)    # [triu | triu]
    bcc = inp("bcc", [128, 128], bf16)        # r64: 1s||0s, r65: 0s||lam
    maskT = None if causal else inp("maskT", [S, S])
    out_ext = nc.dram_tensor("out", [256, S], f32, kind="ExternalOutput")

    with tile.TileContext(nc) as tc:
        with (
            tc.tile_pool(name="const", bufs=1) as cpool,
            tc.tile_pool(name="wts", bufs=1) as wpool,
            tc.tile_pool(name="proj", bufs=1) as ppool,
            tc.tile_pool(name="acts", bufs=2) as apool,
            tc.tile_pool(name="edata", bufs=2) as epool,
            tc.tile_pool(name="small", bufs=2) as spool,
            tc.tile_pool(name="outs", bufs=2) as opool,
            tc.tile_pool(name="dram", bufs=1, space="DRAM") as dpool,
        ):
            # tiny AllGather issued first: absorbs cross-core launch skew
            # while the load phase runs, so real collectives see synced
            # peers.  4-rank (not 8): core 0's CC queue must never wait on
            # the last-dispatched cores 4-7 it shares no data with.
            dummy_in = dpool.tile([1, 16], bf16, name="dummy_in")
            dummy_out = dpool.tile([4, 16], bf16, name="dummy_out")
            nc.gpsimd.dma_start(dummy_in[:], dtriu[0:1, 0:16])
            nc.gpsimd.collective_compute(
                "AllGather", mybir.AluOpType.bypass, replica_groups=RG4,
                ins=[dummy_in.opt()], outs=[dummy_out.opt()])

            # ---- big loads all on the sync DGE, in dependency order ----
            wsb = {}
            for name, t in (("Wq1", Wq1), ("Wq2", Wq2)):
                wsb[name] = wpool.tile([128, 8 * 128], bf16, tag=name,
                                       name=name)
                nc.sync.dma_start(
                    wsb[name][:].rearrange("p (d c) -> p d c", d=8),
                    t.rearrange("(d p) c -> p d c", p=128))
            qblk = []
            for g in range(2):
                qb = apool.tile([128, 4 * S], bf16, tag="qTd",
                                name=f"qblk{g}")
                nc.sync.dma_start(
                    qb[:].rearrange("p (d x) -> p d x", d=4),
                    qT[g * 512:(g + 1) * 512, :]
                    .rearrange("(d p) x -> p d x", p=128))
                qblk.append(qb)
            for name, t in (("Wk1", Wk1), ("Wk2", Wk2)):
                wsb[name] = wpool.tile([128, 8 * 128], bf16, tag=name,
                                       name=name)
                nc.sync.dma_start(
                    wsb[name][:].rearrange("p (d c) -> p d c", d=8),
                    t.rearrange("(d p) c -> p d c", p=128))
            kblk = []
            vtiles = []
            for g in range(2):
                kb = apool.tile([128, 4 * S], bf16, tag="kTd",
                                name=f"kblk{g}")
                nc.sync.dma_start(
                    kb[:].rearrange("p (d x) -> p d x", d=4),
                    kT[g * 512:(g + 1) * 512, :]
                    .rearrange("(d p) x -> p d x", p=128))
                kblk.append(kb)
            for g in range(2):
                vb = apool.tile([128, 4 * S], bf16, tag="vTd",
                                name=f"vblk{g}")
                nc.sync.dma_start(
                    vb[:].rearrange("p (d x) -> p d x", d=4),
                    vT[g * 512:(g + 1) * 512, :]
                    .rearrange("(d p) x -> p d x", p=128))
                vtiles.append(vb)
            wv_sb = wpool.tile([128, 8 * 256], bf16, tag="Wv")
            nc.sync.dma_start(wv_sb[:].rearrange("p (d c) -> p d c", d=8),
                              Wv.rearrange("(d p) c -> p d c", p=128))
            wo_sb = wpool.tile([128, 8 * 256], bf16, tag="Wob")
            nc.sync.dma_start(wo_sb[:].rearrange("p (d c) -> p d c", d=8),
                              Wob.rearrange("(d p) c -> p d c", p=128))

            # ---- small consts on the scalar DGE ----
            dtriu_sb = cpool.tile([128, 256], bf16, tag="dtriu")
            nc.scalar.dma_start(dtriu_sb[:], dtriu[:, :])
            bcc_sb = cpool.tile([128, 128], bf16, tag="bcc")
            nc.scalar.dma_start(bcc_sb[:], bcc[:, :])
            ones1 = cpool.tile([1, 128], bf16, tag="ones1")
            nc.scalar.dma_start(ones1[:], ones_in[:, :])
            bsb = {}
            for name, t in (("bq1", bq1), ("bq2", bq2), ("bk1", bk1),
                            ("bk2", bk2)):
                bsb[name] = cpool.tile([128, 1], f32, tag=name, name=name)
                nc.scalar.dma_start(bsb[name][:], t[:, :])
            bv_sb = cpool.tile([1, 256], bf16, tag="bv")
            nc.scalar.dma_start(bv_sb[:], bv[:, :])

            # ---- phase B: q then k projections ----
            # Each projection lands as TWO [64, S] tiles (heads 0-1, 2-3)
            # so per-head 32-row slices sit at matmul-legal bases {0, 32}.
            def split_tiles(pfx):
                return [ppool.tile([64, S], bf16, tag=f"{pfx}{j}",
                                   name=f"{pfx}{j}") for j in range(2)]
            q1T = split_tiles("q1T"); q2T = split_tiles("q2T")
            k1T = split_tiles("k1T"); k2T = split_tiles("k2T")

            def proj_pair(pool_name, blk, w1, w2, b1, b2, o1T, o2T):
                with tc.tile_pool(name=pool_name, bufs=1,
                                  space="PSUM") as psB:
                    p1 = psB.tile([128, S], f32, tag="p1", name=pool_name + "1")
                    p2 = psB.tile([128, S], f32, tag="p2", name=pool_name + "2")
                    for d in range(8):
                        xTd = blk[d // 4][:, (d % 4) * S:(d % 4 + 1) * S]
                        for ps, wname in ((p1, w1), (p2, w2)):
                            lhsT = wsb[wname][:, d * 128:(d + 1) * 128]
                            for half in range(2):
                                nc.tensor.matmul(
                                    ps[:, half * 512:(half + 1) * 512], lhsT,
                                    xTd[:, half * 512:(half + 1) * 512],
                                    start=(d == 0), stop=(d == 7))
                    # aligned halves on ACT, shifted halves on DVE
                    nc.scalar.activation(o1T[0][:], p1[0:64, :], AF.Identity,
                                         bias=bsb[b1][0:64, :])
                    nc.vector.tensor_scalar(o1T[1][:], p1[64:128, :],
                                            bsb[b1][64:128, :], None, OP.add)
                    nc.scalar.activation(o2T[0][:], p2[0:64, :], AF.Identity,
                                         bias=bsb[b2][0:64, :])
                    nc.vector.tensor_scalar(o2T[1][:], p2[64:128, :],
                                            bsb[b2][64:128, :], None, OP.add)

            # phase C PSUM (4 banks) coexists with the 4-bank proj pools
            # so attention's pools never wait on the v-projection drain.
            vvo = ppool.tile([128, 8 * 260], bf16, tag="vvo")
            with tc.tile_pool(name="psC", bufs=1, space="PSUM") as psC:
                proj_pair("psBq", qblk, "Wq1", "Wq2", "bq1", "bq2", q1T, q2T)
                proj_pair("psBk", kblk, "Wk1", "Wk2", "bk1", "bk2", k1T, k2T)

                # ---- vv projection in two rounds of 4 s-tiles ----
                for rnd in range(2):
                    pr = [psC.tile([128, 256], f32, tag=f"vv{j}",
                                   name=f"vv{rnd}{j}") for j in range(4)]
                    for d in range(8):
                        vTd = vtiles[d // 4][:, (d % 4) * S:(d % 4 + 1) * S]
                        for j in range(4):
                            i = 4 * rnd + j
                            nc.tensor.matmul(
                                pr[j][:], vTd[:, i * 128:(i + 1) * 128],
                                wv_sb[:, d * 256:(d + 1) * 256],
                                start=(d == 0), stop=False)
                    for j in range(4):
                        i = 4 * rnd + j
                        nc.tensor.matmul(pr[j][:], ones1[:], bv_sb[:],
                                         start=False, stop=True)
                        blk = vvo[:, i * 260:(i + 1) * 260]
                        blk3 = blk.rearrange("p (h c) -> p h c", c=65)
                        nc.vector.tensor_copy(
                            blk3[:, :, 0:64],
                            pr[j].rearrange("p (h c) -> p h c", c=64))
                        nc.vector.memset(blk3[:, :, 64:65], 1.0)

            # ---- phase D: attention, chunked over q; AG + Wo per chunk ----
            qproj = (q1T, q2T)
            kproj = (k1T, k2T)
            dtriu3 = dtriu_sb[:].rearrange("p (m x) -> p m x", m=2)
            with (
                tc.tile_pool(name="psS", bufs=2, space="PSUM") as psS,
                tc.tile_pool(name="psO", bufs=2, space="PSUM") as psO,
                tc.tile_pool(name="psP", bufs=1, space="PSUM") as psP,
                tc.tile_pool(name="psW", bufs=1, space="PSUM") as psW,
                tc.tile_pool(name="mloc", bufs=2) as mpool,
            ):
                ship = {}
                for c in (1, 0):
                    cs = c * CHW
                    ilist = list(range(min(NSK, (cs + CHW) // 128))) \
                        if causal else list(range(NSK))
                    if not causal:
                        mT = {}
                        for i in ilist:
                            mT[i] = mpool.tile([128, CHW], f32,
                                               tag=f"mT{i}",
                                               name=f"mT{c}{i}")
                            nc.sync.dma_start(
                                mT[i][:],
                                maskT[i * 128:(i + 1) * 128, cs:cs + CHW])
                    aTt = [spool.tile([128, CHW], bf16, tag=f"aT{kk}",
                                      name=f"aT{kk}c{c}")
                           for kk in range(2)]
                    for h in range(4):
                        hj = h // 2
                        hp = slice(32 * (h % 2), 32 * (h % 2) + 32)
                        etiles = []
                        nx = len(ilist)
                        o0 = psO.tile([128, CHW], f32, tag="o",
                                      name=f"o0{c}{h}")
                        o1 = psO.tile([128, CHW], f32, tag="o",
                                      name=f"o1{c}{h}")

                        def av(x):
                            i, e, off = etiles[x]
                            lhsT = vvo[:, 260 * i + 65 * h:
                                       260 * i + 65 * h + 65]
                            nc.tensor.matmul(
                                o0[0:65, off:CHW], lhsT, e[:, off:CHW],
                                start=(x == 0), stop=(x == nx - 1))
                            nc.tensor.matmul(
                                o1[0:65, off:CHW], lhsT,
                                e[:, CHW + off:2 * CHW],
                                start=(x == 0), stop=(x == nx - 1))

                        for x, i in enumerate(ilist):
                            lo = max(cs, 128 * i) if causal else cs
                            n = cs + CHW - lo
                            off = lo - cs
                            ps = psS.tile([128, 2 * CHW], f32, tag="s",
                                          name=f"s{c}{h}{i}")
                            for m in range(2):
                                nc.tensor.matmul(
                                    ps[:, m * CHW:m * CHW + n],
                                    kproj[m][hj][hp, i * 128:(i + 1) * 128],
                                    qproj[m][hj][hp, lo:cs + CHW],
                                    start=True, stop=True)
                            if not causal:
                                for m in range(2):
                                    nc.vector.tensor_tensor(
                                        ps[:, m * CHW:m * CHW + n],
                                        ps[:, m * CHW:m * CHW + n],
                                        mT[i][:, 0:n], OP.add)
                            e = epool.tile([128, 2 * CHW], bf16,
                                           tag=f"e{i}",
                                           name=f"e{c}h{h}i{i}")
                            e3 = e[:].rearrange("p (m x) -> p m x", m=2)
                            ps3 = ps[:].rearrange("p (m x) -> p m x", m=2)
                            nc.scalar.activation(
                                e3[:, :, off:CHW], ps3[:, :, 0:n], AF.Exp,
                                scale=0.125)
                            if causal and 128 * i >= cs:
                                nc.vector.tensor_tensor(
                                    e3[:, :, off:off + 128],
                                    e3[:, :, off:off + 128],
                                    dtriu3, OP.mult)
                            etiles.append((i, e, off))
                            # AV for the previous tile keeps the PE fed
                            # while ACT runs this tile's exp
                            if x >= 1:
                                av(x - 1)
                        av(nx - 1)
                        # colsum rows -> SBUF (partition-aligned at 64)
                        rs2 = spool.tile([65, 2 * CHW], f32, tag="rs2",
                                         name=f"rs2{c}{h}")
                        nc.scalar.copy(rs2[64:65, 0:CHW], o0[64:65, :])
                        nc.scalar.copy(rs2[64:65, CHW:2 * CHW], o1[64:65, :])
                        # compact reciprocal: rows -> [128, 8] via SBUF DMA,
                        # recip there (recip is ~6.4ns per free elem), back
                        cT = spool.tile([128, 8], f32, tag="cT",
                                        name=f"cT{c}{h}")
                        nc.sync.dma_start(cT[:], rs2[64:65, :])
                        cT2 = spool.tile([128, 8], bf16, tag="cT2",
                                         name=f"cT2{c}{h}")
                        with nc.allow_low_precision(
                                reason="1/colsum as bf16 bcast operand"):
                            nc.vector.reciprocal(cT2[:], cT[:])
                        rrow = spool.tile([66, CHW], bf16, tag="rrow",
                                          name=f"rrow{c}{h}")
                        nc.sync.dma_start(
                            rrow[64:65, :].rearrange("o (p x) -> o p x", p=64),
                            cT2[0:64, :])
                        nc.sync.dma_start(
                            rrow[65:66, :].rearrange("o (p x) -> o p x", p=64),
                            cT2[64:128, :])
                        # single K=2 broadcast: rows 0:64 = 1/s1,
                        # rows 64:128 = lam/s2   (lam folded into bcc row 65)
                        pbc = psP.tile([128, CHW], f32, tag="pbc",
                                       name=f"pbc{c}{h}")
                        nc.tensor.matmul(
                            pbc[:], bcc_sb[64:66, 0:128], rrow[64:66, :],
                            start=True, stop=True)
                        pbcS = spool.tile([128, CHW], f32, tag="pbcS",
                                          name=f"pbcS{c}{h}")
                        nc.vector.tensor_copy(pbcS[:], pbc[:])
                        t0 = spool.tile([64, CHW], f32, tag="t0",
                                        name=f"t0{c}{h}")
                        t1 = spool.tile([64, CHW], f32, tag="t1",
                                        name=f"t1{c}{h}")
                        nc.vector.tensor_tensor(t0[:], o0[0:64, :],
                                                pbcS[0:64, :], OP.mult)
                        nc.vector.tensor_tensor(t1[:], o1[0:64, :],
                                                pbcS[64:128, :], OP.mult)
                        dst = aTt[h // 2][64 * (h % 2):64 * (h % 2) + 64, :]
                        nc.vector.tensor_tensor(dst, t0[:], t1[:],
                                                OP.subtract)
                    # ship this chunk: 4-rank AllGather within batch group
                    bounce = dpool.tile([256, CHW], bf16, name=f"bounce{c}")
                    agc = dpool.tile([CPB * 256, CHW], bf16,
                                     name=f"agc{c}")
                    for kk in range(2):
                        nc.sync.dma_start(
                            bounce[128 * kk:128 * (kk + 1), :], aTt[kk][:])
                    cc = nc.gpsimd.collective_compute(
                        "AllGather", mybir.AluOpType.bypass,
                        replica_groups=RG4,
                        ins=[bounce.opt()], outs=[agc.opt()])
                    ship[c] = (agc, cc, cs)

                # Wo loop AFTER both collectives are issued.  myt/out DMAs
                # go on the gpsimd queue: their long semaphore waits (on the
                # AllGather / Wo copies) must not head-of-line-block the
                # sync queue that carries attention-internal DMAs.
                for c in (1, 0):
                    agc, cc, cs = ship[c]
                    myt = opool.tile([128, 8 * CHW], bf16, tag="myt",
                                     name=f"myt{c}")
                    md = nc.gpsimd.dma_start(
                        myt[:].rearrange("p (k x) -> p k x", k=8),
                        agc.rearrange("(k p) x -> p k x", p=128))
                    add_dep_helper(md.ins, cc.ins, reason="myt after AG")
                    for ch in range(2):
                        wps = psW.tile([128, CHW], f32, tag="wo",
                                       name=f"wo{c}{ch}")
                        for k in range(8):
                            nc.tensor.matmul(
                                wps[:],
                                wo_sb[:, 256 * k + 128 * ch:
                                      256 * k + 128 * ch + 128],
                                myt[:, CHW * k:CHW * (k + 1)],
                                start=(k == 0), stop=(k == 7))
                        osb = opool.tile([128, CHW], f32, tag="osb",
                                         name=f"osb{c}{ch}")
                        if ch == 0:
                            nc.scalar.copy(osb[:], wps[:])
                        else:
                            nc.vector.tensor_copy(osb[:], wps[:])
                        nc.gpsimd.dma_start(
                            out_ext[128 * ch:128 * (ch + 1), cs:cs + CHW],
                            osb[:])

    nc.compile()
    return nc


def kernel(**inputs):
    global LAST_EXEC_NS
    import ml_dtypes

    q = np.asarray(inputs["q"], dtype=np.float32)
    k = np.asarray(inputs["k"], dtype=np.float32)
    v = np.asarray(inputs["v"], dtype=np.float32)
    mask = np.asarray(inputs["mask"])
    f32 = np.float32
    Wq1f = np.asarray(inputs["Wq1"], f32); Wq2f = np.asarray(inputs["Wq2"], f32)
    Wk1f = np.asarray(inputs["Wk1"], f32); Wk2f = np.asarray(inputs["Wk2"], f32)
    Wvf = np.asarray(inputs["Wv"], f32);   Wof = np.asarray(inputs["Wo"], f32)
    bq1f = np.asarray(inputs["bq1"], f32); bq2f = np.asarray(inputs["bq2"], f32)
    bk1f = np.asarray(inputs["bk1"], f32); bk2f = np.asarray(inputs["bk2"], f32)
    bvf = np.asarray(inputs["bv"], f32);   bof = np.asarray(inputs["bo"], f32)
    lam = float(np.exp(float(inputs["lq1"][0]) * float(inputs["lk1"][0]))
                - np.exp(float(inputs["lq2"][0]) * float(inputs["lk2"][0]))
                + LAMBDA_INIT)

    mk = (mask.reshape(B, S, S) != 0)
    causal = bool((mk == np.tril(np.ones((S, S), bool))[None]).all())

    key = "causal" if causal else "general"
    if key not in _cache:
        _cache[key] = _build(causal)
    nc = _cache[key]

    bfl = ml_dtypes.bfloat16
    qT = [np.ascontiguousarray(q[b].T).astype(bfl) for b in range(B)]
    kTl = [np.ascontiguousarray(k[b].T).astype(bfl) for b in range(B)]
    vTl = [np.ascontiguousarray(v[b].T).astype(bfl) for b in range(B)]
    Wob = Wof.astype(bfl)
    triu = np.triu(np.ones((128, 128), f32))
    dtriu = np.concatenate([triu, triu], axis=1).astype(bfl)
    bcc = np.zeros((128, 128), f32)
    bcc[64, 0:64] = 1.0
    bcc[65, 64:128] = lam
    bcc = bcc.astype(ml_dtypes.bfloat16)
    maskTs = None
    if not causal:
        maskTs = [np.ascontiguousarray(
            np.where(mk[b], np.float32(0), np.float32(-1e9)).T)
            for b in range(B)]

    in_maps = []
    for c in range(NCORES):
        b, g = divmod(c, CPB)
        im = dict(
            qT=qT[b], kT=kTl[b], vT=vTl[b],
            Wq1=np.ascontiguousarray(Wq1f[:, 128 * g:128 * (g + 1)]).astype(bfl),
            Wq2=np.ascontiguousarray(Wq2f[:, 128 * g:128 * (g + 1)]).astype(bfl),
            Wk1=np.ascontiguousarray(Wk1f[:, 128 * g:128 * (g + 1)]).astype(bfl),
            Wk2=np.ascontiguousarray(Wk2f[:, 128 * g:128 * (g + 1)]).astype(bfl),
            Wv=np.ascontiguousarray(Wvf[:, 256 * g:256 * (g + 1)]).astype(bfl),
            Wob=np.ascontiguousarray(Wob[:, 256 * g:256 * (g + 1)]),
            bq1=np.ascontiguousarray(bq1f[128 * g:128 * (g + 1)]).reshape(128, 1),
            bq2=np.ascontiguousarray(bq2f[128 * g:128 * (g + 1)]).reshape(128, 1),
            bk1=np.ascontiguousarray(bk1f[128 * g:128 * (g + 1)]).reshape(128, 1),
            bk2=np.ascontiguousarray(bk2f[128 * g:128 * (g + 1)]).reshape(128, 1),
            bv=np.ascontiguousarray(bvf[256 * g:256 * (g + 1)]).reshape(1, 256).astype(bfl),
            dtriu=dtriu, bcc=bcc,
            ones1=np.ones((1, 128), bfl),
        )
        if not causal:
            im["maskT"] = maskTs[b]
        in_maps.append(im)

    from concourse.bass_utils import run_bass_kernel_spmd
    if PROFILE:
        _try_install_ntff_hook()
        res = run_bass_kernel_spmd(nc, in_maps, list(range(NCORES)),
                                   trace=True)
        LAST_EXEC_NS = res.exec_time_ns
        globals()["LAST_RESULTS"] = res
    else:
        res = run_bass_kernel_spmd(nc, in_maps, list(range(NCORES)))

    out = np.empty((B, S, D), np.float32)
    for c in range(NCORES):
        b, g = divmod(c, CPB)
        o = res.results[c]["out"]
        out[b, :, 256 * g:256 * (g + 1)] = o.T
    out += bof[None, None, :]
    return out
